# revision 14
# baseline (speedup 1.0000x reference)
"""CQAttention Trainium2 kernel (8-core data parallel), v5.

Math (per example):
    S[i,j] = C@w_c [i] + Q@w_q [j] + (C*w_mul)@Q^T [i,j] + bias
    S1 = softmax_j(where(Qmask==0, -1e9, S));  S2 = softmax_i(where(Cmask==0, -1e9, S))
    A = S1 @ Q;  Bm = S1 @ S2^T @ C;  out = concat([C, A, C*A, C*Bm], -1)

Structure (device computes only raw numerators/denominators; the host does
all divides, the C*A / C*Bm multiplies, the concat, and the f32 upcast):
  - QM = w_mul*Q^T + w_c and Qneg2 = Q@w_q + qneg are host-precomputed.
  - E1[j,i] = exp(QM^T@CT + Qneg2[j]) -- one matmul+exp orientation serves
    both softmaxes: S1 rows are E1 columns (the s0[i] term rides along and
    cancels); S2 numerator columns are E1 rows (the Qneg2[j] factor cancels
    in the per-column normalization, eps-guarded for fully-masked columns).
  - Et tiles = PE transposes of E1; their PSUM->SBUF evictions (DVE) also
    emit r[i] = sum_j E1[j,i] per tile via accum_out -> one outR store.
  - C-mask folds multiplicatively into the traw rhs (host packs cm*C|cm):
        Traw|c = Et^T @ [cm*C | cm],  T' = Traw * 1/(c+eps)   [j, d] bf16
  - Araw^T and Bmraw^T with the SMALL operand stationary:
        out[d, i] = sum_j W[j,d] * E1[j,i],  W = Qb then T'
    (2 x 512-col matmuls each) -> outAB bf16 [d-major]; host untransposes.

Schedule: 3-deep software pipeline so every cross-engine dependency is
produced a full iteration before its consumer; PE order per iteration k:
tr(k+1) x8, traw(k) x8, e1(k+2) x2, ab(k) x4. HWDGE dma_start costs
~0.6us of sequencer issue time each, so loads are batched into a few
multi-example transfers split across the SP and SWDGE rings, and stores
go on the SP ring (keeping the ACT queue free for compute). Warmup
matmuls ramp the HAM clock gate while the first loads land.
"""

import os
import sys
from contextlib import ExitStack

import ml_dtypes
import numpy as np

for _p in ("/opt/trn_rl_repo", "/root/.axon_site/_ro/trn_rl_repo"):
    if os.path.isdir(_p) and _p not in sys.path:
        sys.path.append(_p)

import concourse.bass as bass
import concourse.tile as tile
from concourse import bacc, mybir
from concourse.bass import ds, ts
from concourse.bass_utils import run_bass_kernel_spmd

F32 = mybir.dt.float32
FP16 = mybir.dt.float16
BF16 = mybir.dt.bfloat16
AF = mybir.ActivationFunctionType
ALU = mybir.AluOpType

N_CORES = 8
B, LC, LQ, D = 64, 1024, 128, 128
B_LOC = B // N_CORES  # 8 examples per core
NT = LC // 128  # 8 Lc tiles of 128
CMW = NT * 130  # packed cmC|cm row length per example


def _build_graph():
    nc = bacc.Bacc("TRN2", target_bir_lowering=False, debug=False)

    CT = nc.dram_tensor("CT", [B_LOC, D, LC], FP16, kind="ExternalInput").ap()
    QM = nc.dram_tensor("QM", [B_LOC, D, LQ], FP16, kind="ExternalInput").ap()
    Qb = nc.dram_tensor("Qb", [B_LOC, LQ, D], BF16, kind="ExternalInput").ap()
    # host-packed, p-major: [p, t*130+x] = (cm*C)[t*128+p, x] | cm | 0
    Cmb = nc.dram_tensor("Cmb", [B_LOC, 128, CMW], BF16, kind="ExternalInput").ap()
    Qneg2 = nc.dram_tensor("Qneg2", [LQ, B_LOC], F32, kind="ExternalInput").ap()
    Ident = nc.dram_tensor("Ident", [128, 128], BF16, kind="ExternalInput").ap()
    # d-major raw output per example: [d, Araw^T(1024) | Bmraw^T(1024)]
    outAB = nc.dram_tensor("outAB", [B_LOC, 128, 2 * LC], BF16, kind="ExternalOutput").ap()
    outR = nc.dram_tensor("outR", [B_LOC, 128, NT], F32, kind="ExternalOutput").ap()

    with tile.TileContext(nc) as tc:
        with ExitStack() as ctx:
            ep = ctx.enter_context

            const = ep(tc.tile_pool(name="const", bufs=1))
            big = ep(tc.tile_pool(name="big", bufs=1))
            p_e1 = ep(tc.tile_pool(name="e1sb", bufs=3))
            p_et = ep(tc.tile_pool(name="et", bufs=3))
            p_tp = ep(tc.tile_pool(name="tp", bufs=3))
            p_stg = ep(tc.tile_pool(name="stg", bufs=2))
            p_small = ep(tc.tile_pool(name="small", bufs=12))

            pp_512 = ep(tc.tile_pool(name="pp512", bufs=5, space="PSUM"))
            pp_tr = ep(tc.tile_pool(name="pp_tr", bufs=2, space="PSUM"))
            pp_traw = ep(tc.tile_pool(name="pp_traw", bufs=1, space="PSUM"))

            # ---- batched loads: few big DMAs, earliest-needed first ----
            # SP (HWDGE) ring: first two examples + small consts
            qm_all = big.tile([128, B_LOC * LQ], FP16, name="qm_all")
            nc.sync.dma_start(qm_all.rearrange("p (e j) -> p e j", j=LQ), QM.rearrange("e p j -> p e j"))
            ct01 = big.tile([128, 2 * LC], FP16, name="ct01")
            nc.sync.dma_start(ct01.rearrange("p (e i) -> p e i", i=LC), CT[0:2].rearrange("e p i -> p e i"))
            qneg2_sb = const.tile([LQ, B_LOC], F32)
            nc.sync.dma_start(qneg2_sb, Qneg2)
            ident_sb = const.tile([128, 128], BF16)
            nc.sync.dma_start(ident_sb, Ident)
            cmb01 = big.tile([128, 2 * CMW], BF16, name="cmb01")
            nc.sync.dma_start(cmb01.rearrange("p (e x) -> p e x", x=CMW), Cmb[0:2].rearrange("e p x -> p e x"))
            # SWDGE (gpsimd) ring: the rest, in consumption order
            qb_all = big.tile([128, B_LOC * D], BF16, name="qb_all")
            nc.gpsimd.dma_start(qb_all.rearrange("j (e d) -> j e d", d=D), Qb.rearrange("e j d -> j e d"))
            ct23 = big.tile([128, 2 * LC], FP16, name="ct23")
            nc.gpsimd.dma_start(ct23.rearrange("p (e i) -> p e i", i=LC), CT[2:4].rearrange("e p i -> p e i"))
            cmb27 = big.tile([128, 6 * CMW], BF16, name="cmb27")
            nc.gpsimd.dma_start(cmb27.rearrange("p (e x) -> p e x", x=CMW), Cmb[2:8].rearrange("e p x -> p e x"))
            ct47 = big.tile([128, 4 * LC], FP16, name="ct47")
            nc.gpsimd.dma_start(ct47.rearrange("p (e i) -> p e i", i=LC), CT[4:8].rearrange("e p i -> p e i"))

            def ct_ap(e):
                if e < 2:
                    return ct01[:, ds(e * LC, LC)]
                if e < 4:
                    return ct23[:, ds((e - 2) * LC, LC)]
                return ct47[:, ds((e - 4) * LC, LC)]

            def cmb_ap(e):
                if e < 2:
                    return cmb01[:, ds(e * CMW, CMW)]
                return cmb27[:, ds((e - 2) * CMW, CMW)]

            def qm_ap(e):
                return qm_all[:, ds(e * LQ, LQ)]

            def qb_ap(e):
                return qb_all[:, ds(e * D, D)]

            # r tiles persist; single store at the end
            rp_all = big.tile([128, B_LOC * NT], F32, name="rp_all")

            # ---- PE warmup while loads land: ramp HAM toward K=8/8 ----
            warm_w = const.tile([128, 512], BF16)
            nc.vector.memset(warm_w, 1.0)
            for w in range(5):
                warm_ps = pp_512.tile([128, 512], F32, tag="p512", name=f"warm_{w}")
                nc.tensor.matmul(warm_ps, lhsT=warm_w[:, 0:128], rhs=warm_w)

            e1_sbs = [None] * B_LOC
            et_sbs = [None] * B_LOC
            tp_sbs = [None] * B_LOC

            def emit_e1(e):
                e1_sb = p_e1.tile([128, LC], BF16, tag="e1sb", name=f"e1_{e}")
                for h in range(2):
                    e1_ps = pp_512.tile([128, 512], F32, tag="p512", name=f"e1ps_{e}_{h}")
                    nc.tensor.matmul(e1_ps, lhsT=qm_ap(e), rhs=ct_ap(e)[:, ts(h, 512)])
                    nc.scalar.activation(
                        e1_sb[:, ts(h, 512)],
                        e1_ps,
                        func=AF.Exp,
                        bias=qneg2_sb[:, e : e + 1],
                        scale=1.0,
                    )
                e1_sbs[e] = e1_sb

            def emit_tr(e):
                # Et = E1^T per 128-tile (PE transpose); DVE evictions also
                # emit r[i] = sum_j E1[j,i] per tile via accum_out.
                et_sb = p_et.tile([128, NT * 128], BF16, tag="et", name=f"et_{e}")
                for t in range(NT):
                    # rotate PSUM banks so the PE write of tile t never shares
                    # a bank with the still-pending eviction read of t-1
                    tr_ps = pp_tr.tile([128, 128], BF16, tag="ptr", name=f"trps_{e}_{t}")
                    nc.tensor.transpose(tr_ps, e1_sbs[e][:, ts(t, 128)], ident_sb)
                    nc.vector.tensor_scalar(
                        et_sb[:, ts(t, 128)],
                        tr_ps,
                        0.0,
                        None,
                        op0=ALU.add,
                        op1=ALU.add,
                        accum_out=rp_all[:, e * NT + t : e * NT + t + 1],
                    )
                et_sbs[e] = et_sb

            def emit_traw(e):
                traw_ps = pp_traw.tile([128, 132], F32, tag="ptraw", name=f"traw_{e}")
                for t in range(NT):
                    nc.tensor.matmul(
                        traw_ps[:, 0:129],
                        lhsT=et_sbs[e][:, ts(t, 128)],
                        rhs=cmb_ap(e)[:, ds(130 * t, 129)],
                        start=(t == 0),
                        stop=(t == NT - 1),
                    )
                ceps = p_small.tile([128, 1], F32, tag="small", name=f"ceps_{e}")
                nc.vector.tensor_scalar_add(ceps, traw_ps[:, 128:129], 1e-30)
                cinv = p_small.tile([128, 1], F32, tag="small", name=f"cinv_{e}")
                nc.vector.reciprocal(cinv, ceps)
                tp_sb = p_tp.tile([128, D], BF16, tag="tp", name=f"tp_{e}")
                nc.vector.tensor_scalar_mul(tp_sb, traw_ps[:, 0:128], cinv)
                tp_sbs[e] = tp_sb

            def emit_ab(e):
                # Araw^T | Bmraw^T: stationary Qb / T', moving E1 (512-col halves)
                stg = p_stg.tile([128, 4, 512], BF16, tag="stg", name=f"stg_{e}")
                for u in range(4):
                    lhsT = qb_ap(e) if u < 2 else tp_sbs[e]
                    ab_ps = pp_512.tile([128, 512], F32, tag="p512", name=f"ab_{e}_{u}")
                    nc.tensor.matmul(
                        ab_ps, lhsT=lhsT, rhs=e1_sbs[e][:, ts(u % 2, 512)]
                    )
                    if u == 1:
                        nc.vector.tensor_copy(stg[:, u, :], ab_ps)
                    else:
                        nc.scalar.activation(stg[:, u, :], ab_ps, func=AF.Copy)
                    if u == 1:
                        nc.sync.dma_start(
                            outAB[e][:, 0:LC].rearrange("p (u x) -> p u x", x=512),
                            stg[:, 0:2, :],
                        )
                nc.sync.dma_start(
                    outAB[e][:, LC : 2 * LC].rearrange("p (u x) -> p u x", x=512),
                    stg[:, 2:4, :],
                )

            # ---- 3-deep software-pipelined main loop ----
            emit_e1(0)
            emit_e1(1)
            emit_tr(0)
            for e in range(B_LOC):
                if e + 1 < B_LOC:
                    emit_tr(e + 1)
                emit_traw(e)
                if e + 2 < B_LOC:
                    emit_e1(e + 2)
                emit_ab(e)
            nc.sync.dma_start(outR.rearrange("e p t -> p e t"), rp_all.rearrange("p (e t) -> p e t", t=NT))

    nc.compile()
    return nc


_GRAPH = None


def _graph():
    global _GRAPH
    if _GRAPH is None:
        _GRAPH = _build_graph()
    return _GRAPH


def make_in_maps(C, Q, Cmask, Qmask, w_c, w_q, w_mul):
    """Shard full inputs into per-core input maps (host-side layout prep)."""
    C = np.asarray(C, dtype=np.float32)
    Q = np.asarray(Q, dtype=np.float32)
    wmul = np.asarray(w_mul, dtype=np.float32).reshape(1, D, 1)
    wc = np.asarray(w_c, dtype=np.float32).reshape(1, D, 1)
    wqv = np.asarray(w_q, dtype=np.float32).reshape(D)
    ident = np.eye(128, dtype=ml_dtypes.bfloat16)
    in_maps = []
    for i in range(N_CORES):
        sl = slice(i * B_LOC, (i + 1) * B_LOC)
        qneg = (np.asarray(Qmask[sl], dtype=np.float32) - 1.0) * 1e9  # [8, 128]
        cm = np.asarray(Cmask[sl], dtype=np.float32)  # [8, 1024]
        Ci = C[sl]
        Qi = Q[sl]
        # QM = w_mul * Q^T + w_c  [8, 128(d), 128(j)]
        qm = (wmul * Qi.transpose(0, 2, 1) + wc).astype(np.float16)
        # Qneg2 = Q@w_q + qneg  [8, 128] -> [128, 8]
        qneg2 = (Qi @ wqv + qneg).astype(np.float32)
        # p-major packed [e, p, t*130+x]
        cmb = np.zeros((B_LOC, LC, 130), dtype=ml_dtypes.bfloat16)
        cmb[:, :, 0:128] = (Ci * cm[:, :, None]).astype(ml_dtypes.bfloat16)
        cmb[:, :, 128] = cm.astype(ml_dtypes.bfloat16)
        cmb = np.ascontiguousarray(
            cmb.reshape(B_LOC, NT, 128, 130)
            .transpose(0, 2, 1, 3)
            .reshape(B_LOC, 128, CMW)
        )
        in_maps.append(
            {
                "CT": np.ascontiguousarray(Ci.transpose(0, 2, 1).astype(np.float16)),
                "QM": np.ascontiguousarray(qm),
                "Qb": np.ascontiguousarray(Qi.astype(ml_dtypes.bfloat16)),
                "Cmb": cmb,
                "Qneg2": np.ascontiguousarray(qneg2.T),  # [128, 8]
                "Ident": ident,
            }
        )
    return in_maps


def assemble(results, C):
    """Gather per-core raw outputs; divide, multiply, concat on host."""
    C = np.asarray(C, dtype=np.float32)
    out = np.empty((B, LC, 4 * D), dtype=np.float32)
    out[:, :, 0:D] = C
    for i in range(N_CORES):
        sl = slice(i * B_LOC, (i + 1) * B_LOC)
        ab = np.asarray(results[i]["outAB"], dtype=np.float32)  # [8, 128(d), 2048]
        at = ab[:, :, 0:LC]  # Araw^T [8, d, i]
        bt = ab[:, :, LC : 2 * LC]  # Bmraw^T
        r = np.asarray(results[i]["outR"], dtype=np.float32)  # [8, 128(p), NT]
        rr = r.transpose(0, 2, 1).reshape(B_LOC, LC, 1)  # [8, i, 1]
        a = at.transpose(0, 2, 1) / rr
        bm = bt.transpose(0, 2, 1) / rr
        Ci = C[sl]
        out[sl, :, D : 2 * D] = a
        out[sl, :, 2 * D : 3 * D] = Ci * a
        out[sl, :, 3 * D : 4 * D] = Ci * bm
    return out


def kernel(C, Q, Cmask, Qmask, w_c, w_q, w_mul, bias=None, **_ignored):
    # `bias` is mathematically a no-op: it shifts every score equally and
    # softmax is shift-invariant, so the output does not depend on it.
    nc = _graph()
    in_maps = make_in_maps(C, Q, Cmask, Qmask, w_c, w_q, w_mul)
    res = run_bass_kernel_spmd(nc, in_maps, core_ids=list(range(N_CORES)))
    return assemble(res.results, C)
